# revision 2
# baseline (speedup 1.0000x reference)
"""MultiHead Differential Attention on 8 Trainium2 NeuronCores — v4.

Sharding: data-parallel over batch (B=2), tensor-parallel over heads
(16 heads -> 4 per core).  Core c handles batch c//4, heads (c%4)*4..+4.

v4 vs v3:
  - chunk-granular, component-packed attention tiles: each S psum tile holds
    [S1|S2] for ONE k-chunk, doubling the sg-rotation pipeline depth (the
    dominant PE stall in v3), one exp instruction per chunk covering both
    components, and component-packed fold adds (half the DVE instructions).
  - filler pieces pack both psum slots: proj QT/KT share one sg tile, two
    outproj oc-chunks share one sg tile (fewer sg rotations contending with
    the S pipeline).
"""
import numpy as np
import ml_dtypes
from contextlib import ExitStack

import concourse.bass as bass
import concourse.mybir as mybir
import concourse.tile as tile
from concourse import bacc
from concourse.bass_utils import run_bass_kernel_spmd

BF16 = mybir.dt.bfloat16
F32 = mybir.dt.float32
AF = mybir.ActivationFunctionType
ALU = mybir.AluOpType

D_MODEL = 1024
H = 16
DH = 64          # head dim per component
HD = 2 * DH      # 128, per-head width of Q/K/V
N = 2048         # sequence length
B = 2
HPC = 4          # heads per core
LAMBDA_INIT = 0.8
EPS = 1e-5
SCALING = 1.0 / np.sqrt(DH)

MC = D_MODEL // 128   # 8 contraction chunks for projections
QC = 4                # q chunks of 512
KCQ = 4               # k-chunks (128) per q chunk
NKC = 16              # total k chunks

_cache = {}


def _patch_act_tables():
    """Force Exp and Ln to resolve to the single set that contains both,
    so alternating Exp/Ln never reloads activation tables."""
    import concourse.bacc as bacc_mod
    import concourse.hw_specs as hw_specs_mod
    if getattr(bacc_mod, "_act_tables_patched", False):
        return
    orig = hw_specs_mod.get_activation_tables

    def patched(arch):
        t = orig(arch)
        for name, fns in t.items():
            if name != "natural_log_exp_and_others":
                fns.discard(AF.Exp)
                fns.discard(AF.Ln)
        return t

    bacc_mod.get_activation_tables = patched
    bacc_mod._act_tables_patched = True


def _build():
    _patch_act_tables()
    nc = bacc.Bacc("TRN2", target_bir_lowering=False, debug=False)

    xt_d = nc.dram_tensor("xt", [128, MC, N], BF16, kind="ExternalInput").ap()
    wq_d = nc.dram_tensor("wq", [128, MC, HPC * HD], BF16, kind="ExternalInput").ap()
    wk_d = nc.dram_tensor("wk", [128, MC, HPC * HD], BF16, kind="ExternalInput").ap()
    wv_d = nc.dram_tensor("wv", [128, MC, HPC * HD], BF16, kind="ExternalInput").ap()
    wo_d = nc.dram_tensor("wo", [128, HPC, 8, 128], BF16, kind="ExternalInput").ap()
    # cols 0..HPC-1: lambda per head; cols HPC..2*HPC-1: 1/max(|lambda|,1)
    lam_d = nc.dram_tensor("lam", [128, 2 * HPC], F32, kind="ExternalInput").ap()
    msk_d = nc.dram_tensor("msk", [128, 1, 128], BF16, kind="ExternalInput").ap()
    out_d = nc.dram_tensor("outT", [D_MODEL, N], BF16, kind="ExternalOutput").ap()

    with tile.TileContext(nc) as tc, ExitStack() as ctx:
        # ---- long-lived tiles
        keep = ctx.enter_context(tc.tile_pool(name="keep", bufs=1))
        qt = [keep.tile([128, N], BF16, tag=f"qt{h}", name=f"qt{h}") for h in range(HPC)]
        kt = [keep.tile([128, N], BF16, tag=f"kt{h}", name=f"kt{h}") for h in range(HPC)]
        vb = keep.tile([128, NKC, 512], BF16, tag="vb")
        otf = [keep.tile([128, N], BF16, tag=f"otf{h}", name=f"otf{h}") for h in range(HPC)]
        lam_t = keep.tile([128, 2 * HPC], F32, tag="lam")
        msk_t = keep.tile([128, 1, 128], BF16, tag="msk")
        ones_t = keep.tile([128, 128], BF16, tag="ones")
        eps_t = keep.tile([128, 1], F32, tag="eps")
        wo_t = keep.tile([128, HPC, 8, 128], BF16, tag="wo")

        nc.gpsimd.memset(ones_t[:], 1.0)
        nc.gpsimd.memset(eps_t[:], float(EPS))

        pj = ctx.enter_context(tc.tile_pool(name="proj", bufs=1))
        psum = ctx.enter_context(tc.tile_pool(name="psum", bufs=1, space="PSUM"))
        at = ctx.enter_context(tc.tile_pool(name="att", bufs=2))
        ep = ctx.enter_context(tc.tile_pool(name="esb", bufs=2))
        osb = ctx.enter_context(tc.tile_pool(name="osb", bufs=2))

        xtb = pj.tile([128, MC, N], BF16, tag="xtb")
        wqb = pj.tile([128, MC, HPC * HD], BF16, tag="wqb")
        wkb = pj.tile([128, MC, HPC * HD], BF16, tag="wkb")
        wvb = pj.tile([128, MC, HPC * HD], BF16, tag="wvb")
        for mc in range(MC):
            nc.sync.dma_start(xtb[:, mc, :], xt_d[:, mc, :])
            nc.sync.dma_start(wvb[:, mc, :], wv_d[:, mc, :])
        for mc in range(MC):
            nc.sync.dma_start(wqb[:, mc, :], wq_d[:, mc, :])
            nc.sync.dma_start(wkb[:, mc, :], wk_d[:, mc, :])
        nc.sync.dma_start(lam_t[:], lam_d[:])
        nc.sync.dma_start(msk_t[:], msk_d[:])
        nc.sync.dma_start(wo_t[:], wo_d[:])

        def proj_v(sc):
            """V rows for seq chunk sc -> vb[:, sc, :]."""
            t_ = psum.tile([128, 2, 512], F32, tag="sg", name="vps", bufs=2)
            ps = t_[:, 0, :]
            for mc in range(MC):
                nc.tensor.matmul(
                    ps[:],
                    xtb[:, mc, sc * 128:(sc + 1) * 128],
                    wvb[:, mc, :],
                    start=(mc == 0), stop=(mc == MC - 1))
            nc.vector.tensor_copy(vb[:, sc, :], ps[:])

        def proj_qk(qc, h):
            """QT and KT for head h, q chunk qc — one sg tile, slot0=Q slot1=K."""
            ps = psum.tile([128, 2, 512], F32, tag="sg", name="qkps", bufs=2)
            for mc in range(MC):
                nc.tensor.matmul(
                    ps[:, 0, :],
                    wqb[:, mc, h * HD:(h + 1) * HD],
                    xtb[:, mc, qc * 512:(qc + 1) * 512],
                    start=(mc == 0), stop=(mc == MC - 1))
                nc.tensor.matmul(
                    ps[:, 1, :],
                    wkb[:, mc, h * HD:(h + 1) * HD],
                    xtb[:, mc, qc * 512:(qc + 1) * 512],
                    start=(mc == 0), stop=(mc == MC - 1))
            nc.vector.tensor_copy(qt[h][:, qc * 512:(qc + 1) * 512], ps[:, 0, :])
            nc.vector.tensor_copy(kt[h][:, qc * 512:(qc + 1) * 512], ps[:, 1, :])

        def outproj2(qc, oc, copy_eng):
            """Output projection for q chunk qc, output-dim chunks oc, oc+1."""
            ps = psum.tile([128, 2, 512], F32, tag="sg", name="ops", bufs=2)
            for h in range(HPC):
                nc.tensor.matmul(
                    ps[:, 0, :], wo_t[:, h, oc, :],
                    otf[h][:, qc * 512:(qc + 1) * 512],
                    start=(h == 0), stop=(h == HPC - 1))
                nc.tensor.matmul(
                    ps[:, 1, :], wo_t[:, h, oc + 1, :],
                    otf[h][:, qc * 512:(qc + 1) * 512],
                    start=(h == 0), stop=(h == HPC - 1))
            ob = osb.tile([128, 2, 512], BF16, tag="ob", bufs=3)
            if copy_eng == 0:
                nc.vector.tensor_copy(
                    ob[:].rearrange("p a b -> p (a b)"),
                    ps[:].rearrange("p a b -> p (a b)"))
            else:
                nc.scalar.copy(
                    ob[:].rearrange("p a b -> p (a b)"),
                    ps[:].rearrange("p a b -> p (a b)"))
            for i in range(2):
                nc.sync.dma_start(
                    out_d[(oc + i) * 128:(oc + i + 1) * 128,
                          qc * 512:(qc + 1) * 512],
                    ob[:, i, :])

        def attn_head(qc, h, filler=None):
            nkc = KCQ * qc + KCQ  # k chunks in play
            q0 = qc * 512
            # s12: slot 0 = row sums of e2, slot 1 = row sums of e1
            # o12: slot 0 = o1 (P1 @ V),  slot 1 = o2 (P2 @ V)
            s12 = psum.tile([128, 2, 512], F32, tag="s12")
            o12 = psum.tile([128, 2, 512], F32, tag="o12")
            pending = []
            pair_hold = [None]   # e-tile of an even full chunk awaiting its partner
            fold_hold = [None]   # pair-sum awaiting its partner pair
            ones_first = [True]

            def ones_mm(src_pair, last):
                """src_pair: [128, 2, 512] (slot0 ~ e1 fold, slot1 ~ e2 fold)."""
                st = ones_first[0]
                ones_first[0] = False
                nc.tensor.matmul(s12[:, 1, :], ones_t[:], src_pair[:, 0, :],
                                 start=st, stop=last)
                nc.tensor.matmul(s12[:, 0, :], ones_t[:], src_pair[:, 1, :],
                                 start=st, stop=last)

            def ones_mm_diag(e, w0, st, last):
                nc.tensor.matmul(s12[:, 1, w0:512], ones_t[:], e[:, 0, w0:512],
                                 start=st, stop=last)
                nc.tensor.matmul(s12[:, 0, w0:512], ones_t[:], e[:, 1, w0:512],
                                 start=st, stop=last)

            def emit_chunk(item):
                e, kc = item
                j = kc - KCQ * qc
                st = (kc == 0)
                sp = (kc == nkc - 1)
                if j < 0:
                    # full chunk: fold pairs -> quads, one ones-MM per quad
                    if pair_hold[0] is None:
                        pair_hold[0] = e
                    else:
                        p = ep.tile([128, 2, 512], BF16, tag="pf", name="pf",
                                    bufs=3)
                        nc.vector.tensor_add(
                            p[:].rearrange("p a b -> p (a b)"),
                            pair_hold[0][:].rearrange("p a b -> p (a b)"),
                            e[:].rearrange("p a b -> p (a b)"))
                        pair_hold[0] = None
                        if fold_hold[0] is None and kc + 2 < KCQ * qc:
                            fold_hold[0] = p
                        elif fold_hold[0] is not None:
                            qd = ep.tile([128, 2, 512], BF16, tag="qf",
                                         name="qf", bufs=2)
                            nc.vector.tensor_add(
                                qd[:].rearrange("p a b -> p (a b)"),
                                fold_hold[0][:].rearrange("p a b -> p (a b)"),
                                p[:].rearrange("p a b -> p (a b)"))
                            fold_hold[0] = None
                            ones_mm(qd, last=False)
                        else:
                            ones_mm(p, last=False)
                else:
                    # diagonal chunk: causal mask then per-chunk ones MMs
                    w0 = 128 * j
                    mskb = msk_t[:, :, :].broadcast_to((128, 2, 128))
                    nc.vector.tensor_mul(
                        e[:, :, w0:w0 + 128], e[:, :, w0:w0 + 128], mskb)
                    ones_mm_diag(e, w0, st, sp)
                w0 = max(0, 128 * j)
                nc.tensor.matmul(
                    o12[:, 0, w0:512], vb[:, kc, h * HD:(h + 1) * HD],
                    e[:, 0, w0:512], start=st, stop=sp)
                nc.tensor.matmul(
                    o12[:, 1, w0:512], vb[:, kc, h * HD:(h + 1) * HD],
                    e[:, 1, w0:512], start=st, stop=sp)

            for kc in range(nkc):
                j = kc - KCQ * qc
                w0 = max(0, 128 * j)
                sch = psum.tile([128, 2, 512], F32, tag="sg", name="sch",
                                bufs=2)
                nc.tensor.matmul(
                    sch[:, 0, w0:512], kt[h][0:64, kc * 128:(kc + 1) * 128],
                    qt[h][0:64, q0 + w0:q0 + 512], start=True, stop=True)
                nc.tensor.matmul(
                    sch[:, 1, w0:512], kt[h][64:128, kc * 128:(kc + 1) * 128],
                    qt[h][64:128, q0 + w0:q0 + 512], start=True, stop=True)
                e = ep.tile([128, 2, 512], BF16, tag="e", name="e", bufs=7)
                nc.scalar.activation(
                    e[:, :, w0:512], sch[:, :, w0:512],
                    AF.Exp, scale=float(SCALING))
                pending.append((e, kc))
                if len(pending) > 3:
                    emit_chunk(pending.pop(0))
            while pending:
                emit_chunk(pending.pop(0))

            # ---- epilogue: d = (o1*s2 - lam*o2*s1)/max(|lam|,1) (up to a
            # per-column scale that the channel RMS norm washes out; output
            # sign is folded into Wo on the host).  The 1/g factor keeps ssq
            # inside ScalarE's Ln domain when lam is huge.  tensor_tensor can
            # read only one PSUM operand, so s12 is staged through SBUF with
            # the 1/g scale folded in.
            s12s = at.tile([128, 2, 512], F32, tag="s12s")
            nc.vector.tensor_scalar(
                s12s[:].rearrange("p a b -> p (a b)"),
                s12[:].rearrange("p a b -> p (a b)"),
                lam_t[:, HPC + h:HPC + h + 1], None, ALU.mult)
            m = at.tile([128, 2, 512], F32, tag="m")
            nc.vector.tensor_mul(m[:], o12[:], s12s[:])
            dneg = at.tile([128, 512], BF16, tag="dneg")
            nc.vector.scalar_tensor_tensor(
                dneg[:], m[:, 1, :], lam_t[:, h:h + 1], m[:, 0, :],
                ALU.mult, ALU.subtract)
            osq = at.tile([128, 512], BF16, tag="osq")
            nc.vector.tensor_mul(osq[:], dneg[:], dneg[:])
            if filler is not None:
                filler()
            ssq = psum.tile([128, 512], F32, tag="s12", name="ssq")
            nc.tensor.matmul(ssq[:], ones_t[:], osq[:], start=True, stop=True)
            lnv = at.tile([128, 512], F32, tag="lnv")
            nc.scalar.activation(lnv[:], ssq[:], AF.Ln,
                                 scale=float(1.0 / HD), bias=eps_t[:])
            rr = at.tile([128, 512], BF16, tag="rr")
            nc.scalar.activation(rr[:], lnv[:], AF.Exp, scale=-0.5)
            nc.vector.tensor_mul(otf[h][:, q0:q0 + 512], dneg[:], rr[:])

        # ================= pipelined schedule =================
        for sc in range(4):
            proj_v(sc)
        for h in range(HPC):
            proj_qk(0, h)

        for qc in range(QC):
            for h in range(HPC):
                def filler(qc=qc, h=h):
                    if qc + 1 < QC:
                        proj_v(4 * (qc + 1) + h)
                        proj_qk(qc + 1, h)
                    if qc >= 1:
                        outproj2(qc - 1, 2 * h, copy_eng=h % 2)
                attn_head(qc, h, filler)
        for oc in range(0, 8, 2):
            outproj2(QC - 1, oc, copy_eng=(oc // 2) % 2)

    nc.compile()
    return nc


def _prep_inputs(X, Wq, Wk, Wv, Wo, lambda_q1, lambda_k1, lambda_q2,
                 lambda_k2, rms_scale):
    f32 = np.float32
    bf16 = ml_dtypes.bfloat16
    X = np.asarray(X, f32)
    Wq = np.asarray(Wq, f32)
    Wk = np.asarray(Wk, f32)
    Wv = np.asarray(Wv, f32)
    Wo = np.asarray(Wo, f32)
    lam = (np.exp(np.sum(np.asarray(lambda_q1, f32) * np.asarray(lambda_k1, f32), -1))
           - np.exp(np.sum(np.asarray(lambda_q2, f32) * np.asarray(lambda_k2, f32), -1))
           + f32(LAMBDA_INIT)).astype(f32)  # [H]
    # fold rms_scale, (1-lambda_init), and the epilogue sign flip into Wo
    wo_f = (-(Wo.reshape(H, HD, D_MODEL)
              * np.asarray(rms_scale, f32)[None, :, None]
              * f32(1.0 - LAMBDA_INIT))).astype(f32)

    # upper-triangle causal mask for diagonal 128-blocks
    kk = np.arange(128)[:, None]
    cc = np.arange(128)[None, :]
    msk = (cc >= kk).astype(f32).reshape(128, 1, 128)

    in_maps = []
    for c in range(8):
        b, hg = divmod(c, 4)
        xt = X[b].T.reshape(MC, 128, N).transpose(1, 0, 2)  # [128, MC, N]
        sl = slice(hg * HPC * HD, (hg + 1) * HPC * HD)
        wq = Wq[:, sl].reshape(MC, 128, HPC * HD).transpose(1, 0, 2)
        wk = Wk[:, sl].reshape(MC, 128, HPC * HD).transpose(1, 0, 2)
        wv = Wv[:, sl].reshape(MC, 128, HPC * HD).transpose(1, 0, 2)
        wo = wo_f[hg * HPC:(hg + 1) * HPC].reshape(HPC, HD, 8, 128).transpose(1, 0, 2, 3)
        lv = lam[hg * HPC:(hg + 1) * HPC]
        ginv = (f32(1.0) / np.maximum(np.abs(lv), f32(1.0))).astype(f32)
        lam_row = np.concatenate([lv, ginv]).astype(f32)
        lam_bc = np.broadcast_to(lam_row[None, :], (128, 2 * HPC))
        in_maps.append({
            "xt": np.ascontiguousarray(xt).astype(bf16),
            "wq": np.ascontiguousarray(wq).astype(bf16),
            "wk": np.ascontiguousarray(wk).astype(bf16),
            "wv": np.ascontiguousarray(wv).astype(bf16),
            "wo": np.ascontiguousarray(wo).astype(bf16),
            "lam": np.ascontiguousarray(lam_bc).astype(f32),
            "msk": msk.astype(bf16),
        })
    return in_maps


def kernel(X, Wq, Wk, Wv, Wo, lambda_q1, lambda_k1, lambda_q2, lambda_k2,
           rms_scale, _trace=False):
    if "nc" not in _cache:
        _cache["nc"] = _build()
    nc = _cache["nc"]
    in_maps = _prep_inputs(X, Wq, Wk, Wv, Wo, lambda_q1, lambda_k1,
                           lambda_q2, lambda_k2, rms_scale)
    res = run_bass_kernel_spmd(nc, in_maps, list(range(8)), trace=_trace)
    out = np.zeros((B, N, D_MODEL), np.float32)
    for c in range(8):
        b = c // 4
        out[b] += res.results[c]["outT"].astype(np.float32).T
    _cache["last_exec_ns"] = res.exec_time_ns
    _cache["last_res"] = res
    return out


# revision 3
# speedup vs baseline: 1.0066x; 1.0066x over previous
"""MultiHead Differential Attention on 8 Trainium2 NeuronCores — v6.

Sharding: data-parallel over batch (B=2), tensor-parallel over heads
(16 heads -> 4 per core).  Core c handles batch c//4, heads (c%4)*4..+4.

v6 vs v4:
  - filler pieces (proj for qc+1, outproj for qc-1) are interleaved INSIDE
    the S-chunk loop so their psum drains complete mid-head instead of
    queueing behind the epilogue at the head boundary.
  - each head's epilogue finale (ssq matmul + Ln/Exp + otf multiply) is
    deferred into the NEXT head's S-stream, removing the ssq stall; s12/o12
    are allocated lazily so the psum tag rotation stays consistent.
  - diagonal chunks fold into the j=0 tile (one ones-MM pair per head
    instead of four).
"""
import numpy as np
import ml_dtypes
from contextlib import ExitStack

import concourse.bass as bass
import concourse.mybir as mybir
import concourse.tile as tile
from concourse import bacc
from concourse.bass_utils import run_bass_kernel_spmd

BF16 = mybir.dt.bfloat16
F32 = mybir.dt.float32
AF = mybir.ActivationFunctionType
ALU = mybir.AluOpType

D_MODEL = 1024
H = 16
DH = 64          # head dim per component
HD = 2 * DH      # 128, per-head width of Q/K/V
N = 2048         # sequence length
B = 2
HPC = 4          # heads per core
LAMBDA_INIT = 0.8
EPS = 1e-5
SCALING = 1.0 / np.sqrt(DH)

MC = D_MODEL // 128   # 8 contraction chunks for projections
QC = 4                # q chunks of 512
KCQ = 4               # k-chunks (128) per q chunk
NKC = 16              # total k chunks

_cache = {}


def _patch_act_tables():
    """Force Exp and Ln to resolve to the single set that contains both,
    so alternating Exp/Ln never reloads activation tables."""
    import concourse.bacc as bacc_mod
    import concourse.hw_specs as hw_specs_mod
    if getattr(bacc_mod, "_act_tables_patched", False):
        return
    orig = hw_specs_mod.get_activation_tables

    def patched(arch):
        t = orig(arch)
        for name, fns in t.items():
            if name != "natural_log_exp_and_others":
                fns.discard(AF.Exp)
                fns.discard(AF.Ln)
        return t

    bacc_mod.get_activation_tables = patched
    bacc_mod._act_tables_patched = True


def _build():
    _patch_act_tables()
    nc = bacc.Bacc("TRN2", target_bir_lowering=False, debug=False)

    xt_d = nc.dram_tensor("xt", [128, MC, N], BF16, kind="ExternalInput").ap()
    wq_d = nc.dram_tensor("wq", [128, MC, HPC * HD], BF16, kind="ExternalInput").ap()
    wk_d = nc.dram_tensor("wk", [128, MC, HPC * HD], BF16, kind="ExternalInput").ap()
    wv_d = nc.dram_tensor("wv", [128, MC, HPC * HD], BF16, kind="ExternalInput").ap()
    wo_d = nc.dram_tensor("wo", [128, HPC, 8, 128], BF16, kind="ExternalInput").ap()
    # cols 0..HPC-1: lambda per head; cols HPC..2*HPC-1: 1/max(|lambda|,1)
    lam_d = nc.dram_tensor("lam", [128, 2 * HPC], F32, kind="ExternalInput").ap()
    msk_d = nc.dram_tensor("msk", [128, 1, 128], BF16, kind="ExternalInput").ap()
    out_d = nc.dram_tensor("outT", [D_MODEL, N], BF16, kind="ExternalOutput").ap()

    with tile.TileContext(nc) as tc, ExitStack() as ctx:
        # ---- long-lived tiles
        keep = ctx.enter_context(tc.tile_pool(name="keep", bufs=1))
        qt = [keep.tile([128, N], BF16, tag=f"qt{h}", name=f"qt{h}") for h in range(HPC)]
        kt = [keep.tile([128, N], BF16, tag=f"kt{h}", name=f"kt{h}") for h in range(HPC)]
        vb = keep.tile([128, NKC, 512], BF16, tag="vb")
        otf = [keep.tile([128, N], BF16, tag=f"otf{h}", name=f"otf{h}") for h in range(HPC)]
        lam_t = keep.tile([128, 2 * HPC], F32, tag="lam")
        msk_t = keep.tile([128, 1, 128], BF16, tag="msk")
        ones_t = keep.tile([128, 128], BF16, tag="ones")
        eps_t = keep.tile([128, 1], F32, tag="eps")
        wo_t = keep.tile([128, HPC, 8, 128], BF16, tag="wo")

        nc.gpsimd.memset(ones_t[:], 1.0)
        nc.gpsimd.memset(eps_t[:], float(EPS))

        pj = ctx.enter_context(tc.tile_pool(name="proj", bufs=1))
        psum = ctx.enter_context(tc.tile_pool(name="psum", bufs=1, space="PSUM"))
        at = ctx.enter_context(tc.tile_pool(name="att", bufs=2))
        ep = ctx.enter_context(tc.tile_pool(name="esb", bufs=2))
        osb = ctx.enter_context(tc.tile_pool(name="osb", bufs=2))

        xtb = pj.tile([128, MC, N], BF16, tag="xtb")
        wqb = pj.tile([128, MC, HPC * HD], BF16, tag="wqb")
        wkb = pj.tile([128, MC, HPC * HD], BF16, tag="wkb")
        wvb = pj.tile([128, MC, HPC * HD], BF16, tag="wvb")
        for mc in range(MC):
            nc.sync.dma_start(xtb[:, mc, :], xt_d[:, mc, :])
            nc.sync.dma_start(wvb[:, mc, :], wv_d[:, mc, :])
            nc.sync.dma_start(wqb[:, mc, :], wq_d[:, mc, :])
            nc.sync.dma_start(wkb[:, mc, :], wk_d[:, mc, :])
        nc.sync.dma_start(lam_t[:], lam_d[:])
        nc.sync.dma_start(msk_t[:], msk_d[:])
        nc.sync.dma_start(wo_t[:], wo_d[:])

        def proj_v(sc):
            """V rows for seq chunk sc -> vb[:, sc, :]."""
            t_ = psum.tile([128, 2, 512], F32, tag="sg", name="vps", bufs=2)
            ps = t_[:, 0, :]
            for mc in range(MC):
                nc.tensor.matmul(
                    ps[:],
                    xtb[:, mc, sc * 128:(sc + 1) * 128],
                    wvb[:, mc, :],
                    start=(mc == 0), stop=(mc == MC - 1))
            nc.vector.tensor_copy(vb[:, sc, :], ps[:])

        def proj_qk(qc, h):
            """QT and KT for head h, q chunk qc — one sg tile, slot0=Q slot1=K."""
            ps = psum.tile([128, 2, 512], F32, tag="sg", name="qkps", bufs=2)
            for mc in range(MC):
                nc.tensor.matmul(
                    ps[:, 0, :],
                    wqb[:, mc, h * HD:(h + 1) * HD],
                    xtb[:, mc, qc * 512:(qc + 1) * 512],
                    start=(mc == 0), stop=(mc == MC - 1))
                nc.tensor.matmul(
                    ps[:, 1, :],
                    wkb[:, mc, h * HD:(h + 1) * HD],
                    xtb[:, mc, qc * 512:(qc + 1) * 512],
                    start=(mc == 0), stop=(mc == MC - 1))
            nc.vector.tensor_copy(qt[h][:, qc * 512:(qc + 1) * 512], ps[:, 0, :])
            nc.vector.tensor_copy(kt[h][:, qc * 512:(qc + 1) * 512], ps[:, 1, :])

        def outproj2(qc, oc, copy_eng):
            """Output projection for q chunk qc, output-dim chunks oc, oc+1."""
            ps = psum.tile([128, 2, 512], F32, tag="sg", name="ops", bufs=2)
            for h in range(HPC):
                nc.tensor.matmul(
                    ps[:, 0, :], wo_t[:, h, oc, :],
                    otf[h][:, qc * 512:(qc + 1) * 512],
                    start=(h == 0), stop=(h == HPC - 1))
                nc.tensor.matmul(
                    ps[:, 1, :], wo_t[:, h, oc + 1, :],
                    otf[h][:, qc * 512:(qc + 1) * 512],
                    start=(h == 0), stop=(h == HPC - 1))
            ob = osb.tile([128, 2, 512], BF16, tag="ob", bufs=3)
            if copy_eng == 0:
                nc.vector.tensor_copy(
                    ob[:].rearrange("p a b -> p (a b)"),
                    ps[:].rearrange("p a b -> p (a b)"))
            else:
                nc.scalar.copy(
                    ob[:].rearrange("p a b -> p (a b)"),
                    ps[:].rearrange("p a b -> p (a b)"))
            for i in range(2):
                nc.sync.dma_start(
                    out_d[(oc + i) * 128:(oc + i + 1) * 128,
                          qc * 512:(qc + 1) * 512],
                    ob[:, i, :])

        def attn_head(qc, h, finale_prev=None, fillers=()):
            """Emits S/exp/fold/ones/PV and the DVE part of the epilogue for
            head (qc, h).  Returns a finale closure (ssq matmul + Ln/Exp +
            otf multiply) that the caller emits inside the NEXT head."""
            nkc = KCQ * qc + KCQ  # k chunks in play
            q0 = qc * 512
            state = {"s12": None, "o12": None}
            pending = []
            pair_hold = [None]
            fold_hold = [None]
            diag_acc = [None]
            ones_first = [True]

            def get_s12():
                if state["s12"] is None:
                    # slot 0 = row sums of e2, slot 1 = row sums of e1
                    state["s12"] = psum.tile([128, 2, 512], F32, tag="s12",
                                             name="s12")
                return state["s12"]

            def get_o12():
                if state["o12"] is None:
                    # slot 0 = o1 (P1 @ V), slot 1 = o2 (P2 @ V)
                    state["o12"] = psum.tile([128, 2, 512], F32, tag="o12",
                                             name="o12")
                return state["o12"]

            def ones_mm(src_pair, last):
                s12 = get_s12()
                st = ones_first[0]
                ones_first[0] = False
                nc.tensor.matmul(s12[:, 1, :], ones_t[:], src_pair[:, 0, :],
                                 start=st, stop=last)
                nc.tensor.matmul(s12[:, 0, :], ones_t[:], src_pair[:, 1, :],
                                 start=st, stop=last)

            def emit_chunk(item):
                e, kc = item
                j = kc - KCQ * qc
                st = (kc == 0)
                sp = (kc == nkc - 1)
                o12 = get_o12()
                if j < 0:
                    # full chunk: fold pairs -> quads, one ones-MM per quad
                    if pair_hold[0] is None:
                        pair_hold[0] = e
                    else:
                        p = ep.tile([128, 2, 512], BF16, tag="pf", name="pf",
                                    bufs=3)
                        nc.vector.tensor_add(
                            p[:].rearrange("p a b -> p (a b)"),
                            pair_hold[0][:].rearrange("p a b -> p (a b)"),
                            e[:].rearrange("p a b -> p (a b)"))
                        pair_hold[0] = None
                        if fold_hold[0] is None and kc + 2 < KCQ * qc:
                            fold_hold[0] = p
                        elif fold_hold[0] is not None:
                            qd = ep.tile([128, 2, 512], BF16, tag="qf",
                                         name="qf", bufs=2)
                            nc.vector.tensor_add(
                                qd[:].rearrange("p a b -> p (a b)"),
                                fold_hold[0][:].rearrange("p a b -> p (a b)"),
                                p[:].rearrange("p a b -> p (a b)"))
                            fold_hold[0] = None
                            ones_mm(qd, last=False)
                        else:
                            ones_mm(p, last=False)
                else:
                    # diagonal chunk: causal mask, fold into the j=0 tile,
                    # single ones-MM pair once all four are in
                    w0 = 128 * j
                    mskb = msk_t[:, :, :].broadcast_to((128, 2, 128))
                    nc.vector.tensor_mul(
                        e[:, :, w0:w0 + 128], e[:, :, w0:w0 + 128], mskb)
                    if diag_acc[0] is None:
                        diag_acc[0] = e
                    else:
                        a = diag_acc[0]
                        nc.vector.tensor_add(
                            a[:, :, w0:512], a[:, :, w0:512], e[:, :, w0:512])
                    if sp:
                        ones_mm(diag_acc[0], last=True)
                w0 = max(0, 128 * j)
                nc.tensor.matmul(
                    o12[:, 0, w0:512], vb[:, kc, h * HD:(h + 1) * HD],
                    e[:, 0, w0:512], start=st, stop=sp)
                nc.tensor.matmul(
                    o12[:, 1, w0:512], vb[:, kc, h * HD:(h + 1) * HD],
                    e[:, 1, w0:512], start=st, stop=sp)

            finale_pos = min(4, nkc - 1)
            fill_at = {}
            for i, f in enumerate(fillers):
                fill_at.setdefault(max(1, (nkc * (i + 1)) // 4), []).append(f)

            for kc in range(nkc):
                j = kc - KCQ * qc
                w0 = max(0, 128 * j)
                sch = psum.tile([128, 2, 512], F32, tag="sg", name="sch",
                                bufs=2)
                nc.tensor.matmul(
                    sch[:, 0, w0:512], kt[h][0:64, kc * 128:(kc + 1) * 128],
                    qt[h][0:64, q0 + w0:q0 + 512], start=True, stop=True)
                nc.tensor.matmul(
                    sch[:, 1, w0:512], kt[h][64:128, kc * 128:(kc + 1) * 128],
                    qt[h][64:128, q0 + w0:q0 + 512], start=True, stop=True)
                e = ep.tile([128, 2, 512], BF16, tag="e", name="e", bufs=7)
                nc.scalar.activation(
                    e[:, :, w0:512], sch[:, :, w0:512],
                    AF.Exp, scale=float(SCALING))
                pending.append((e, kc))
                if finale_prev is not None and kc == finale_pos:
                    finale_prev()
                    finale_prev = None
                for f in fill_at.get(kc, ()):
                    f()
                if len(pending) > 3:
                    emit_chunk(pending.pop(0))
            if finale_prev is not None:
                finale_prev()
            while pending:
                emit_chunk(pending.pop(0))

            # ---- epilogue (DVE part): d = (o1*s2 - lam*o2*s1)/max(|lam|,1)
            # (up to a per-column scale that the channel RMS norm washes out;
            # output sign is folded into Wo on the host).  The 1/g factor
            # keeps ssq inside ScalarE's Ln domain when lam is huge.
            # tensor_tensor can read only one PSUM operand, so s12 is staged
            # through SBUF with the 1/g scale folded in.
            s12, o12 = get_s12(), get_o12()
            s12s = at.tile([128, 2, 512], F32, tag="s12s")
            nc.vector.tensor_scalar(
                s12s[:].rearrange("p a b -> p (a b)"),
                s12[:].rearrange("p a b -> p (a b)"),
                lam_t[:, HPC + h:HPC + h + 1], None, ALU.mult)
            m = at.tile([128, 2, 512], F32, tag="m")
            nc.vector.tensor_mul(m[:], o12[:], s12s[:])
            dneg = at.tile([128, 512], BF16, tag="dneg")
            nc.vector.scalar_tensor_tensor(
                dneg[:], m[:, 1, :], lam_t[:, h:h + 1], m[:, 0, :],
                ALU.mult, ALU.subtract)
            osq = at.tile([128, 512], BF16, tag="osq")
            nc.vector.tensor_mul(osq[:], dneg[:], dneg[:])

            def finale():
                ssq = psum.tile([128, 512], F32, tag="s12", name="ssq")
                nc.tensor.matmul(ssq[:], ones_t[:], osq[:],
                                 start=True, stop=True)
                lnv = at.tile([128, 512], F32, tag="lnv")
                nc.scalar.activation(lnv[:], ssq[:], AF.Ln,
                                     scale=float(1.0 / HD), bias=eps_t[:])
                rr = at.tile([128, 512], BF16, tag="rr")
                nc.scalar.activation(rr[:], lnv[:], AF.Exp, scale=-0.5)
                nc.vector.tensor_mul(otf[h][:, q0:q0 + 512], dneg[:], rr[:])

            return finale

        # ================= pipelined schedule =================
        for sc in range(4):
            proj_v(sc)
        for h in range(HPC):
            proj_qk(0, h)

        finale_prev = None
        for qc in range(QC):
            for h in range(HPC):
                fillers = []
                if qc + 1 < QC:
                    fillers.append(lambda qc=qc, h=h: proj_v(4 * (qc + 1) + h))
                    fillers.append(lambda qc=qc, h=h: proj_qk(qc + 1, h))
                if qc >= 1:
                    fillers.append(
                        lambda qc=qc, h=h: outproj2(qc - 1, 2 * h,
                                                    copy_eng=h % 2))
                finale_prev = attn_head(qc, h, finale_prev, fillers)
        finale_prev()
        for oc in range(0, 8, 2):
            outproj2(QC - 1, oc, copy_eng=(oc // 2) % 2)

    nc.compile()
    return nc


def _prep_inputs(X, Wq, Wk, Wv, Wo, lambda_q1, lambda_k1, lambda_q2,
                 lambda_k2, rms_scale):
    f32 = np.float32
    bf16 = ml_dtypes.bfloat16
    X = np.asarray(X, f32)
    Wq = np.asarray(Wq, f32)
    Wk = np.asarray(Wk, f32)
    Wv = np.asarray(Wv, f32)
    Wo = np.asarray(Wo, f32)
    lam = (np.exp(np.sum(np.asarray(lambda_q1, f32) * np.asarray(lambda_k1, f32), -1))
           - np.exp(np.sum(np.asarray(lambda_q2, f32) * np.asarray(lambda_k2, f32), -1))
           + f32(LAMBDA_INIT)).astype(f32)  # [H]
    # fold rms_scale, (1-lambda_init), and the epilogue sign flip into Wo
    wo_f = (-(Wo.reshape(H, HD, D_MODEL)
              * np.asarray(rms_scale, f32)[None, :, None]
              * f32(1.0 - LAMBDA_INIT))).astype(f32)

    # upper-triangle causal mask for diagonal 128-blocks
    kk = np.arange(128)[:, None]
    cc = np.arange(128)[None, :]
    msk = (cc >= kk).astype(f32).reshape(128, 1, 128)

    in_maps = []
    for c in range(8):
        b, hg = divmod(c, 4)
        xt = X[b].T.reshape(MC, 128, N).transpose(1, 0, 2)  # [128, MC, N]
        sl = slice(hg * HPC * HD, (hg + 1) * HPC * HD)
        wq = Wq[:, sl].reshape(MC, 128, HPC * HD).transpose(1, 0, 2)
        wk = Wk[:, sl].reshape(MC, 128, HPC * HD).transpose(1, 0, 2)
        wv = Wv[:, sl].reshape(MC, 128, HPC * HD).transpose(1, 0, 2)
        wo = wo_f[hg * HPC:(hg + 1) * HPC].reshape(HPC, HD, 8, 128).transpose(1, 0, 2, 3)
        lv = lam[hg * HPC:(hg + 1) * HPC]
        ginv = (f32(1.0) / np.maximum(np.abs(lv), f32(1.0))).astype(f32)
        lam_row = np.concatenate([lv, ginv]).astype(f32)
        lam_bc = np.broadcast_to(lam_row[None, :], (128, 2 * HPC))
        in_maps.append({
            "xt": np.ascontiguousarray(xt).astype(bf16),
            "wq": np.ascontiguousarray(wq).astype(bf16),
            "wk": np.ascontiguousarray(wk).astype(bf16),
            "wv": np.ascontiguousarray(wv).astype(bf16),
            "wo": np.ascontiguousarray(wo).astype(bf16),
            "lam": np.ascontiguousarray(lam_bc).astype(f32),
            "msk": msk.astype(bf16),
        })
    return in_maps


def kernel(X, Wq, Wk, Wv, Wo, lambda_q1, lambda_k1, lambda_q2, lambda_k2,
           rms_scale, _trace=False):
    if "nc" not in _cache:
        _cache["nc"] = _build()
    nc = _cache["nc"]
    in_maps = _prep_inputs(X, Wq, Wk, Wv, Wo, lambda_q1, lambda_k1,
                           lambda_q2, lambda_k2, rms_scale)
    res = run_bass_kernel_spmd(nc, in_maps, list(range(8)), trace=_trace)
    out = np.zeros((B, N, D_MODEL), np.float32)
    for c in range(8):
        b = c // 4
        out[b] += res.results[c]["outT"].astype(np.float32).T
    _cache["last_exec_ns"] = res.exec_time_ns
    _cache["last_res"] = res
    return out


# revision 4
# speedup vs baseline: 1.0104x; 1.0038x over previous
"""MultiHead Differential Attention on 8 Trainium2 NeuronCores.

Sharding: data-parallel over batch (B=2), tensor-parallel over heads
(16 heads -> 4 per core).  Core c handles batch c//4, heads (c%4)*4..+4.

Fully software-pipelined schedule:
  - projection pieces for qc+1 and output-projection pieces for qc-1 are
    interleaved INSIDE each head's S-chunk loop, so every engine overlaps.
  - chunk-granular, component-packed psum tiles ([S1|S2] per k-chunk) give
    the S pipeline 2-deep rotation; one exp instruction per chunk.
  - epilogue: d = (o1*s2 - lam*o2*s1)/max(|lam|,1) — the per-column scale
    washes out in the channel RMS norm; sign folded into Wo; the finale
    (ssq/Ln/Exp/otf) of each head is deferred into the next head's stream.
  - diagonal chunks fold into one masked accumulator (one ones-MM pair).
  - output stored bf16; host accumulates partial sums in f32.
"""
import numpy as np
import ml_dtypes
from contextlib import ExitStack

import concourse.bass as bass
import concourse.mybir as mybir
import concourse.tile as tile
from concourse import bacc
from concourse.bass_utils import run_bass_kernel_spmd

BF16 = mybir.dt.bfloat16
F32 = mybir.dt.float32
AF = mybir.ActivationFunctionType
ALU = mybir.AluOpType

D_MODEL = 1024
H = 16
DH = 64          # head dim per component
HD = 2 * DH      # 128, per-head width of Q/K/V
N = 2048         # sequence length
B = 2
HPC = 4          # heads per core
LAMBDA_INIT = 0.8
EPS = 1e-5
SCALING = 1.0 / np.sqrt(DH)

MC = D_MODEL // 128   # 8 contraction chunks for projections
QC = 4                # q chunks of 512
KCQ = 4               # k-chunks (128) per q chunk
NKC = 16              # total k chunks

_cache = {}


def _patch_act_tables():
    """Force Exp and Ln to resolve to the single set that contains both,
    so alternating Exp/Ln never reloads activation tables."""
    import concourse.bacc as bacc_mod
    import concourse.hw_specs as hw_specs_mod
    if getattr(bacc_mod, "_act_tables_patched", False):
        return
    orig = hw_specs_mod.get_activation_tables

    def patched(arch):
        t = orig(arch)
        for name, fns in t.items():
            if name != "natural_log_exp_and_others":
                fns.discard(AF.Exp)
                fns.discard(AF.Ln)
        return t

    bacc_mod.get_activation_tables = patched
    bacc_mod._act_tables_patched = True


def _build():
    _patch_act_tables()
    nc = bacc.Bacc("TRN2", target_bir_lowering=False, debug=False)

    xt_d = nc.dram_tensor("xt", [128, MC, N], BF16, kind="ExternalInput").ap()
    wq_d = nc.dram_tensor("wq", [128, MC, HPC * HD], BF16, kind="ExternalInput").ap()
    wk_d = nc.dram_tensor("wk", [128, MC, HPC * HD], BF16, kind="ExternalInput").ap()
    wv_d = nc.dram_tensor("wv", [128, MC, HPC * HD], BF16, kind="ExternalInput").ap()
    wo_d = nc.dram_tensor("wo", [128, HPC, 8, 128], BF16, kind="ExternalInput").ap()
    # cols 0..HPC-1: lambda per head; cols HPC..2*HPC-1: 1/max(|lambda|,1)
    lam_d = nc.dram_tensor("lam", [128, 2 * HPC], F32, kind="ExternalInput").ap()
    msk_d = nc.dram_tensor("msk", [128, 1, 128], BF16, kind="ExternalInput").ap()
    out_d = nc.dram_tensor("outT", [D_MODEL, N], BF16, kind="ExternalOutput").ap()

    with tile.TileContext(nc) as tc, ExitStack() as ctx:
        # ---- long-lived tiles
        keep = ctx.enter_context(tc.tile_pool(name="keep", bufs=1))
        qt = [keep.tile([128, N], BF16, tag=f"qt{h}", name=f"qt{h}") for h in range(HPC)]
        kt = [keep.tile([128, N], BF16, tag=f"kt{h}", name=f"kt{h}") for h in range(HPC)]
        vb = keep.tile([128, NKC, 512], BF16, tag="vb")
        otf = [keep.tile([128, N], BF16, tag=f"otf{h}", name=f"otf{h}") for h in range(HPC)]
        lam_t = keep.tile([128, 2 * HPC], F32, tag="lam")
        msk_t = keep.tile([128, 1, 128], BF16, tag="msk")
        ones_t = keep.tile([128, 128], BF16, tag="ones")
        eps_t = keep.tile([128, 1], F32, tag="eps")
        wo_t = keep.tile([128, HPC, 8, 128], BF16, tag="wo")

        nc.gpsimd.memset(ones_t[:], 1.0)
        nc.gpsimd.memset(eps_t[:], float(EPS))

        pj = ctx.enter_context(tc.tile_pool(name="proj", bufs=1))
        psum = ctx.enter_context(tc.tile_pool(name="psum", bufs=1, space="PSUM"))
        at = ctx.enter_context(tc.tile_pool(name="att", bufs=2))
        ep = ctx.enter_context(tc.tile_pool(name="esb", bufs=2))
        osb = ctx.enter_context(tc.tile_pool(name="osb", bufs=2))

        xtb = pj.tile([128, MC, N], BF16, tag="xtb")
        wqb = pj.tile([128, MC, HPC * HD], BF16, tag="wqb")
        wkb = pj.tile([128, MC, HPC * HD], BF16, tag="wkb")
        wvb = pj.tile([128, MC, HPC * HD], BF16, tag="wvb")
        for mc in range(MC):
            nc.sync.dma_start(xtb[:, mc, :], xt_d[:, mc, :])
            nc.sync.dma_start(wvb[:, mc, :], wv_d[:, mc, :])
            nc.sync.dma_start(wqb[:, mc, :], wq_d[:, mc, :])
            nc.sync.dma_start(wkb[:, mc, :], wk_d[:, mc, :])
        nc.sync.dma_start(lam_t[:], lam_d[:])
        nc.sync.dma_start(msk_t[:], msk_d[:])
        nc.sync.dma_start(wo_t[:], wo_d[:])

        def proj_v(sc):
            """V rows for seq chunk sc -> vb[:, sc, :]."""
            t_ = psum.tile([128, 2, 512], F32, tag="sg", name="vps", bufs=2)
            ps = t_[:, 0, :]
            for mc in range(MC):
                nc.tensor.matmul(
                    ps[:],
                    xtb[:, mc, sc * 128:(sc + 1) * 128],
                    wvb[:, mc, :],
                    start=(mc == 0), stop=(mc == MC - 1))
            nc.vector.tensor_copy(vb[:, sc, :], ps[:])

        def proj_qk(qc, h):
            """QT and KT for head h, q chunk qc — one sg tile, slot0=Q slot1=K."""
            ps = psum.tile([128, 2, 512], F32, tag="sg", name="qkps", bufs=2)
            for mc in range(MC):
                nc.tensor.matmul(
                    ps[:, 0, :],
                    wqb[:, mc, h * HD:(h + 1) * HD],
                    xtb[:, mc, qc * 512:(qc + 1) * 512],
                    start=(mc == 0), stop=(mc == MC - 1))
                nc.tensor.matmul(
                    ps[:, 1, :],
                    wkb[:, mc, h * HD:(h + 1) * HD],
                    xtb[:, mc, qc * 512:(qc + 1) * 512],
                    start=(mc == 0), stop=(mc == MC - 1))
            nc.vector.tensor_copy(qt[h][:, qc * 512:(qc + 1) * 512], ps[:, 0, :])
            nc.vector.tensor_copy(kt[h][:, qc * 512:(qc + 1) * 512], ps[:, 1, :])

        def outproj2(qc, oc, copy_eng):
            """Output projection for q chunk qc, output-dim chunks oc, oc+1."""
            ps = psum.tile([128, 2, 512], F32, tag="sg", name="ops", bufs=2)
            for h in range(HPC):
                nc.tensor.matmul(
                    ps[:, 0, :], wo_t[:, h, oc, :],
                    otf[h][:, qc * 512:(qc + 1) * 512],
                    start=(h == 0), stop=(h == HPC - 1))
                nc.tensor.matmul(
                    ps[:, 1, :], wo_t[:, h, oc + 1, :],
                    otf[h][:, qc * 512:(qc + 1) * 512],
                    start=(h == 0), stop=(h == HPC - 1))
            ob = osb.tile([128, 2, 512], BF16, tag="ob", bufs=3)
            if copy_eng == 0:
                nc.vector.tensor_copy(
                    ob[:].rearrange("p a b -> p (a b)"),
                    ps[:].rearrange("p a b -> p (a b)"))
            else:
                nc.scalar.copy(
                    ob[:].rearrange("p a b -> p (a b)"),
                    ps[:].rearrange("p a b -> p (a b)"))
            for i in range(2):
                nc.sync.dma_start(
                    out_d[(oc + i) * 128:(oc + i + 1) * 128,
                          qc * 512:(qc + 1) * 512],
                    ob[:, i, :])

        def attn_head(qc, h, finale_prev=None, fillers=()):
            """Emits S/exp/fold/ones/PV and the DVE part of the epilogue for
            head (qc, h).  Returns a finale closure (ssq matmul + Ln/Exp +
            otf multiply) that the caller emits inside the NEXT head."""
            nkc = KCQ * qc + KCQ  # k chunks in play
            q0 = qc * 512
            state = {"s12": None, "o12": None}
            pending = []
            pair_hold = [None]
            fold_hold = [None]
            diag_acc = [None]
            ones_first = [True]

            def get_s12():
                if state["s12"] is None:
                    # slot 0 = row sums of e2, slot 1 = row sums of e1
                    state["s12"] = psum.tile([128, 2, 512], F32, tag="s12",
                                             name="s12")
                return state["s12"]

            def get_o12():
                if state["o12"] is None:
                    # slot 0 = o1 (P1 @ V), slot 1 = o2 (P2 @ V)
                    state["o12"] = psum.tile([128, 2, 512], F32, tag="o12",
                                             name="o12")
                return state["o12"]

            def ones_mm(src_pair, last):
                s12 = get_s12()
                st = ones_first[0]
                ones_first[0] = False
                nc.tensor.matmul(s12[:, 1, :], ones_t[:], src_pair[:, 0, :],
                                 start=st, stop=last)
                nc.tensor.matmul(s12[:, 0, :], ones_t[:], src_pair[:, 1, :],
                                 start=st, stop=last)

            def emit_chunk(item):
                e, kc = item
                j = kc - KCQ * qc
                st = (kc == 0)
                sp = (kc == nkc - 1)
                o12 = get_o12()
                if j < 0:
                    # full chunk: fold pairs -> quads, one ones-MM per quad
                    if pair_hold[0] is None:
                        pair_hold[0] = e
                    else:
                        p = ep.tile([128, 2, 512], BF16, tag="pf", name="pf",
                                    bufs=3)
                        nc.vector.tensor_add(
                            p[:].rearrange("p a b -> p (a b)"),
                            pair_hold[0][:].rearrange("p a b -> p (a b)"),
                            e[:].rearrange("p a b -> p (a b)"))
                        pair_hold[0] = None
                        if fold_hold[0] is None and kc + 2 < KCQ * qc:
                            fold_hold[0] = p
                        elif fold_hold[0] is not None:
                            qd = ep.tile([128, 2, 512], BF16, tag="qf",
                                         name="qf", bufs=2)
                            nc.vector.tensor_add(
                                qd[:].rearrange("p a b -> p (a b)"),
                                fold_hold[0][:].rearrange("p a b -> p (a b)"),
                                p[:].rearrange("p a b -> p (a b)"))
                            fold_hold[0] = None
                            ones_mm(qd, last=False)
                        else:
                            ones_mm(p, last=False)
                else:
                    # diagonal chunk: causal mask, fold into the j=0 tile,
                    # single ones-MM pair once all four are in
                    w0 = 128 * j
                    mskb = msk_t[:, :, :].broadcast_to((128, 2, 128))
                    nc.vector.tensor_mul(
                        e[:, :, w0:w0 + 128], e[:, :, w0:w0 + 128], mskb)
                    if diag_acc[0] is None:
                        diag_acc[0] = e
                    else:
                        a = diag_acc[0]
                        nc.vector.tensor_add(
                            a[:, :, w0:512], a[:, :, w0:512], e[:, :, w0:512])
                    if sp:
                        ones_mm(diag_acc[0], last=True)
                w0 = max(0, 128 * j)
                nc.tensor.matmul(
                    o12[:, 0, w0:512], vb[:, kc, h * HD:(h + 1) * HD],
                    e[:, 0, w0:512], start=st, stop=sp)
                nc.tensor.matmul(
                    o12[:, 1, w0:512], vb[:, kc, h * HD:(h + 1) * HD],
                    e[:, 1, w0:512], start=st, stop=sp)

            finale_pos = min(4, nkc - 1)
            fill_at = {}
            for i, f in enumerate(fillers):
                fill_at.setdefault(max(1, (nkc * (i + 1)) // 4), []).append(f)

            for kc in range(nkc):
                j = kc - KCQ * qc
                w0 = max(0, 128 * j)
                sch = psum.tile([128, 2, 512], F32, tag="sg", name="sch",
                                bufs=2)
                nc.tensor.matmul(
                    sch[:, 0, w0:512], kt[h][0:64, kc * 128:(kc + 1) * 128],
                    qt[h][0:64, q0 + w0:q0 + 512], start=True, stop=True)
                nc.tensor.matmul(
                    sch[:, 1, w0:512], kt[h][64:128, kc * 128:(kc + 1) * 128],
                    qt[h][64:128, q0 + w0:q0 + 512], start=True, stop=True)
                e = ep.tile([128, 2, 512], BF16, tag="e", name="e", bufs=7)
                nc.scalar.activation(
                    e[:, :, w0:512], sch[:, :, w0:512],
                    AF.Exp, scale=float(SCALING))
                pending.append((e, kc))
                if finale_prev is not None and kc == finale_pos:
                    finale_prev()
                    finale_prev = None
                for f in fill_at.get(kc, ()):
                    f()
                if len(pending) > 4:
                    emit_chunk(pending.pop(0))
            if finale_prev is not None:
                finale_prev()
            while pending:
                emit_chunk(pending.pop(0))

            # ---- epilogue (DVE part): d = (o1*s2 - lam*o2*s1)/max(|lam|,1)
            # (up to a per-column scale that the channel RMS norm washes out;
            # output sign is folded into Wo on the host).  The 1/g factor
            # keeps ssq inside ScalarE's Ln domain when lam is huge.
            # tensor_tensor can read only one PSUM operand, so s12 is staged
            # through SBUF with the 1/g scale folded in.
            s12, o12 = get_s12(), get_o12()
            s12s = at.tile([128, 2, 512], F32, tag="s12s")
            nc.vector.tensor_scalar(
                s12s[:].rearrange("p a b -> p (a b)"),
                s12[:].rearrange("p a b -> p (a b)"),
                lam_t[:, HPC + h:HPC + h + 1], None, ALU.mult)
            m = at.tile([128, 2, 512], F32, tag="m")
            nc.vector.tensor_mul(m[:], o12[:], s12s[:])
            dneg = at.tile([128, 512], BF16, tag="dneg")
            nc.vector.scalar_tensor_tensor(
                dneg[:], m[:, 1, :], lam_t[:, h:h + 1], m[:, 0, :],
                ALU.mult, ALU.subtract)
            osq = at.tile([128, 512], BF16, tag="osq")
            nc.vector.tensor_mul(osq[:], dneg[:], dneg[:])

            def finale():
                ssq = psum.tile([128, 512], F32, tag="s12", name="ssq")
                nc.tensor.matmul(ssq[:], ones_t[:], osq[:],
                                 start=True, stop=True)
                lnv = at.tile([128, 512], F32, tag="lnv")
                nc.scalar.activation(lnv[:], ssq[:], AF.Ln,
                                     scale=float(1.0 / HD), bias=eps_t[:])
                rr = at.tile([128, 512], BF16, tag="rr")
                nc.scalar.activation(rr[:], lnv[:], AF.Exp, scale=-0.5)
                nc.vector.tensor_mul(otf[h][:, q0:q0 + 512], dneg[:], rr[:])

            return finale

        # ================= pipelined schedule =================
        for sc in range(4):
            proj_v(sc)
        proj_qk(0, 0)

        finale_prev = None
        for qc in range(QC):
            for h in range(HPC):
                fillers = []
                if qc == 0 and h + 1 < HPC:
                    fillers.append(lambda h=h: proj_qk(0, h + 1))
                if qc + 1 < QC:
                    fillers.append(lambda qc=qc, h=h: proj_v(4 * (qc + 1) + h))
                    fillers.append(lambda qc=qc, h=h: proj_qk(qc + 1, h))
                if qc >= 1:
                    fillers.append(
                        lambda qc=qc, h=h: outproj2(qc - 1, 2 * h,
                                                    copy_eng=h % 2))
                finale_prev = attn_head(qc, h, finale_prev, fillers)
        finale_prev()
        for oc in range(0, 8, 2):
            outproj2(QC - 1, oc, copy_eng=(oc // 2) % 2)

    nc.compile()
    return nc


def _prep_inputs(X, Wq, Wk, Wv, Wo, lambda_q1, lambda_k1, lambda_q2,
                 lambda_k2, rms_scale):
    f32 = np.float32
    bf16 = ml_dtypes.bfloat16
    X = np.asarray(X, f32)
    Wq = np.asarray(Wq, f32)
    Wk = np.asarray(Wk, f32)
    Wv = np.asarray(Wv, f32)
    Wo = np.asarray(Wo, f32)
    lam = (np.exp(np.sum(np.asarray(lambda_q1, f32) * np.asarray(lambda_k1, f32), -1))
           - np.exp(np.sum(np.asarray(lambda_q2, f32) * np.asarray(lambda_k2, f32), -1))
           + f32(LAMBDA_INIT)).astype(f32)  # [H]
    # fold rms_scale, (1-lambda_init), and the epilogue sign flip into Wo
    wo_f = (-(Wo.reshape(H, HD, D_MODEL)
              * np.asarray(rms_scale, f32)[None, :, None]
              * f32(1.0 - LAMBDA_INIT))).astype(f32)

    # upper-triangle causal mask for diagonal 128-blocks
    kk = np.arange(128)[:, None]
    cc = np.arange(128)[None, :]
    msk = (cc >= kk).astype(f32).reshape(128, 1, 128)

    in_maps = []
    for c in range(8):
        b, hg = divmod(c, 4)
        xt = X[b].T.reshape(MC, 128, N).transpose(1, 0, 2)  # [128, MC, N]
        sl = slice(hg * HPC * HD, (hg + 1) * HPC * HD)
        wq = Wq[:, sl].reshape(MC, 128, HPC * HD).transpose(1, 0, 2)
        wk = Wk[:, sl].reshape(MC, 128, HPC * HD).transpose(1, 0, 2)
        wv = Wv[:, sl].reshape(MC, 128, HPC * HD).transpose(1, 0, 2)
        wo = wo_f[hg * HPC:(hg + 1) * HPC].reshape(HPC, HD, 8, 128).transpose(1, 0, 2, 3)
        lv = lam[hg * HPC:(hg + 1) * HPC]
        ginv = (f32(1.0) / np.maximum(np.abs(lv), f32(1.0))).astype(f32)
        lam_row = np.concatenate([lv, ginv]).astype(f32)
        lam_bc = np.broadcast_to(lam_row[None, :], (128, 2 * HPC))
        in_maps.append({
            "xt": np.ascontiguousarray(xt).astype(bf16),
            "wq": np.ascontiguousarray(wq).astype(bf16),
            "wk": np.ascontiguousarray(wk).astype(bf16),
            "wv": np.ascontiguousarray(wv).astype(bf16),
            "wo": np.ascontiguousarray(wo).astype(bf16),
            "lam": np.ascontiguousarray(lam_bc).astype(f32),
            "msk": msk.astype(bf16),
        })
    return in_maps


def kernel(X, Wq, Wk, Wv, Wo, lambda_q1, lambda_k1, lambda_q2, lambda_k2,
           rms_scale, _trace=False):
    if "nc" not in _cache:
        _cache["nc"] = _build()
    nc = _cache["nc"]
    in_maps = _prep_inputs(X, Wq, Wk, Wv, Wo, lambda_q1, lambda_k1,
                           lambda_q2, lambda_k2, rms_scale)
    res = run_bass_kernel_spmd(nc, in_maps, list(range(8)), trace=_trace)
    out = np.zeros((B, N, D_MODEL), np.float32)
    for c in range(8):
        b = c // 4
        out[b] += res.results[c]["outT"].astype(np.float32).T
    _cache["last_exec_ns"] = res.exec_time_ns
    _cache["last_res"] = res
    return out


# revision 5
# speedup vs baseline: 1.0240x; 1.0135x over previous
"""MultiHead Differential Attention on 8 Trainium2 NeuronCores.

Sharding: data-parallel over batch (B=2), tensor-parallel over heads
(16 heads -> 4 per core).  Core c handles batch c//4, heads (c%4)*4..+4.

Fully software-pipelined schedule:
  - mc-major prologue: V and all four heads' Q/K projections accumulate
    4-8 matmuls per arriving input-DMA chunk (using the still-idle s12/o12
    psum banks), so PE tracks the input load instead of serializing per
    target.
  - projection pieces for qc+1 and output-projection pieces for qc-1 are
    interleaved INSIDE each head's S-chunk loop, so every engine overlaps.
  - chunk-granular, component-packed psum tiles ([S1|S2] per k-chunk) give
    the S pipeline 2-deep rotation; one exp instruction per chunk.
  - epilogue: d = (o1*s2 - lam*o2*s1)/max(|lam|,1) — the per-column scale
    washes out in the channel RMS norm; sign folded into Wo; each head's
    finale (ssq/Ln/Exp/otf) is deferred into the next head's stream.
  - diagonal chunks fold into one masked accumulator (one ones-MM pair).
  - output stored bf16; host accumulates partial sums in f32.
"""
import numpy as np
import ml_dtypes
from contextlib import ExitStack

import concourse.bass as bass
import concourse.mybir as mybir
import concourse.tile as tile
from concourse import bacc
from concourse.bass_utils import run_bass_kernel_spmd

BF16 = mybir.dt.bfloat16
F32 = mybir.dt.float32
AF = mybir.ActivationFunctionType
ALU = mybir.AluOpType

D_MODEL = 1024
H = 16
DH = 64          # head dim per component
HD = 2 * DH      # 128, per-head width of Q/K/V
N = 2048         # sequence length
B = 2
HPC = 4          # heads per core
LAMBDA_INIT = 0.8
EPS = 1e-5
SCALING = 1.0 / np.sqrt(DH)

MC = D_MODEL // 128   # 8 contraction chunks for projections
QC = 4                # q chunks of 512
KCQ = 4               # k-chunks (128) per q chunk
NKC = 16              # total k chunks

_cache = {}


def _patch_act_tables():
    """Force Exp and Ln to resolve to the single set that contains both,
    so alternating Exp/Ln never reloads activation tables."""
    import concourse.bacc as bacc_mod
    import concourse.hw_specs as hw_specs_mod
    if getattr(bacc_mod, "_act_tables_patched", False):
        return
    orig = hw_specs_mod.get_activation_tables

    def patched(arch):
        t = orig(arch)
        for name, fns in t.items():
            if name != "natural_log_exp_and_others":
                fns.discard(AF.Exp)
                fns.discard(AF.Ln)
        return t

    bacc_mod.get_activation_tables = patched
    bacc_mod._act_tables_patched = True


def _build():
    _patch_act_tables()
    nc = bacc.Bacc("TRN2", target_bir_lowering=False, debug=False)

    xt_d = nc.dram_tensor("xt", [128, MC, N], BF16, kind="ExternalInput").ap()
    wq_d = nc.dram_tensor("wq", [128, MC, HPC * HD], BF16, kind="ExternalInput").ap()
    wk_d = nc.dram_tensor("wk", [128, MC, HPC * HD], BF16, kind="ExternalInput").ap()
    wv_d = nc.dram_tensor("wv", [128, MC, HPC * HD], BF16, kind="ExternalInput").ap()
    wo_d = nc.dram_tensor("wo", [128, HPC, 8, 128], BF16, kind="ExternalInput").ap()
    # cols 0..HPC-1: lambda per head; cols HPC..2*HPC-1: 1/max(|lambda|,1)
    lam_d = nc.dram_tensor("lam", [128, 2 * HPC], F32, kind="ExternalInput").ap()
    msk_d = nc.dram_tensor("msk", [128, 1, 128], BF16, kind="ExternalInput").ap()
    out_d = nc.dram_tensor("outT", [D_MODEL, N], BF16, kind="ExternalOutput").ap()

    with tile.TileContext(nc) as tc, ExitStack() as ctx:
        # ---- long-lived tiles
        keep = ctx.enter_context(tc.tile_pool(name="keep", bufs=1))
        qt = [keep.tile([128, N], BF16, tag=f"qt{h}", name=f"qt{h}") for h in range(HPC)]
        kt = [keep.tile([128, N], BF16, tag=f"kt{h}", name=f"kt{h}") for h in range(HPC)]
        vb = keep.tile([128, NKC, 512], BF16, tag="vb")
        otf = [keep.tile([128, N], BF16, tag=f"otf{h}", name=f"otf{h}") for h in range(HPC)]
        lam_t = keep.tile([128, 2 * HPC], F32, tag="lam")
        msk_t = keep.tile([128, 1, 128], BF16, tag="msk")
        ones_t = keep.tile([128, 128], BF16, tag="ones")
        eps_t = keep.tile([128, 1], F32, tag="eps")
        wo_t = keep.tile([128, HPC, 8, 128], BF16, tag="wo")

        nc.gpsimd.memset(ones_t[:], 1.0)
        nc.gpsimd.memset(eps_t[:], float(EPS))

        pj = ctx.enter_context(tc.tile_pool(name="proj", bufs=1))
        psum = ctx.enter_context(tc.tile_pool(name="psum", bufs=1, space="PSUM"))
        at = ctx.enter_context(tc.tile_pool(name="att", bufs=2))
        ep = ctx.enter_context(tc.tile_pool(name="esb", bufs=2))
        osb = ctx.enter_context(tc.tile_pool(name="osb", bufs=2))

        xtb = pj.tile([128, MC, N], BF16, tag="xtb")
        wqb = pj.tile([128, MC, HPC * HD], BF16, tag="wqb")
        wkb = pj.tile([128, MC, HPC * HD], BF16, tag="wkb")
        wvb = pj.tile([128, MC, HPC * HD], BF16, tag="wvb")
        for mc in range(MC):
            nc.sync.dma_start(xtb[:, mc, :], xt_d[:, mc, :])
            nc.sync.dma_start(wvb[:, mc, :], wv_d[:, mc, :])
            nc.sync.dma_start(wqb[:, mc, :], wq_d[:, mc, :])
            nc.sync.dma_start(wkb[:, mc, :], wk_d[:, mc, :])
        nc.sync.dma_start(lam_t[:], lam_d[:])
        nc.sync.dma_start(msk_t[:], msk_d[:])
        nc.sync.dma_start(wo_t[:], wo_d[:])

        def proj_v(sc):
            """V rows for seq chunk sc -> vb[:, sc, :]."""
            t_ = psum.tile([128, 2, 512], F32, tag="sg", name="vps", bufs=2)
            ps = t_[:, 0, :]
            for mc in range(MC):
                nc.tensor.matmul(
                    ps[:],
                    xtb[:, mc, sc * 128:(sc + 1) * 128],
                    wvb[:, mc, :],
                    start=(mc == 0), stop=(mc == MC - 1))
            nc.vector.tensor_copy(vb[:, sc, :], ps[:])

        def proj_qk(qc, h):
            """QT and KT for head h, q chunk qc — one sg tile, slot0=Q slot1=K."""
            ps = psum.tile([128, 2, 512], F32, tag="sg", name="qkps", bufs=2)
            for mc in range(MC):
                nc.tensor.matmul(
                    ps[:, 0, :],
                    wqb[:, mc, h * HD:(h + 1) * HD],
                    xtb[:, mc, qc * 512:(qc + 1) * 512],
                    start=(mc == 0), stop=(mc == MC - 1))
                nc.tensor.matmul(
                    ps[:, 1, :],
                    wkb[:, mc, h * HD:(h + 1) * HD],
                    xtb[:, mc, qc * 512:(qc + 1) * 512],
                    start=(mc == 0), stop=(mc == MC - 1))
            nc.vector.tensor_copy(qt[h][:, qc * 512:(qc + 1) * 512], ps[:, 0, :])
            nc.vector.tensor_copy(kt[h][:, qc * 512:(qc + 1) * 512], ps[:, 1, :])

        def outproj2(qc, oc, copy_eng):
            """Output projection for q chunk qc, output-dim chunks oc, oc+1."""
            ps = psum.tile([128, 2, 512], F32, tag="sg", name="ops", bufs=2)
            for h in range(HPC):
                nc.tensor.matmul(
                    ps[:, 0, :], wo_t[:, h, oc, :],
                    otf[h][:, qc * 512:(qc + 1) * 512],
                    start=(h == 0), stop=(h == HPC - 1))
                nc.tensor.matmul(
                    ps[:, 1, :], wo_t[:, h, oc + 1, :],
                    otf[h][:, qc * 512:(qc + 1) * 512],
                    start=(h == 0), stop=(h == HPC - 1))
            ob = osb.tile([128, 2, 512], BF16, tag="ob", bufs=3)
            if copy_eng == 0:
                nc.vector.tensor_copy(
                    ob[:].rearrange("p a b -> p (a b)"),
                    ps[:].rearrange("p a b -> p (a b)"))
            else:
                nc.scalar.copy(
                    ob[:].rearrange("p a b -> p (a b)"),
                    ps[:].rearrange("p a b -> p (a b)"))
            for i in range(2):
                nc.sync.dma_start(
                    out_d[(oc + i) * 128:(oc + i + 1) * 128,
                          qc * 512:(qc + 1) * 512],
                    ob[:, i, :])

        def attn_head(qc, h, finale_prev=None, fillers=()):
            """Emits S/exp/fold/ones/PV and the DVE part of the epilogue for
            head (qc, h).  Returns a finale closure (ssq matmul + Ln/Exp +
            otf multiply) that the caller emits inside the NEXT head."""
            nkc = KCQ * qc + KCQ  # k chunks in play
            q0 = qc * 512
            state = {"s12": None, "o12": None}
            pending = []
            pair_hold = [None]
            fold_hold = [None]
            diag_acc = [None]
            ones_first = [True]

            def get_s12():
                if state["s12"] is None:
                    # slot 0 = row sums of e2, slot 1 = row sums of e1
                    state["s12"] = psum.tile([128, 2, 512], F32, tag="s12",
                                             name="s12")
                return state["s12"]

            def get_o12():
                if state["o12"] is None:
                    # slot 0 = o1 (P1 @ V), slot 1 = o2 (P2 @ V)
                    state["o12"] = psum.tile([128, 2, 512], F32, tag="o12",
                                             name="o12")
                return state["o12"]

            def ones_mm(src_pair, last):
                s12 = get_s12()
                st = ones_first[0]
                ones_first[0] = False
                nc.tensor.matmul(s12[:, 1, :], ones_t[:], src_pair[:, 0, :],
                                 start=st, stop=last)
                nc.tensor.matmul(s12[:, 0, :], ones_t[:], src_pair[:, 1, :],
                                 start=st, stop=last)

            def emit_chunk(item):
                e, kc = item
                j = kc - KCQ * qc
                st = (kc == 0)
                sp = (kc == nkc - 1)
                o12 = get_o12()
                if j < 0:
                    # full chunk: fold pairs -> quads, one ones-MM per quad
                    if pair_hold[0] is None:
                        pair_hold[0] = e
                    else:
                        p = ep.tile([128, 2, 512], BF16, tag="pf", name="pf",
                                    bufs=3)
                        nc.vector.tensor_add(
                            p[:].rearrange("p a b -> p (a b)"),
                            pair_hold[0][:].rearrange("p a b -> p (a b)"),
                            e[:].rearrange("p a b -> p (a b)"))
                        pair_hold[0] = None
                        if fold_hold[0] is None and kc + 2 < KCQ * qc:
                            fold_hold[0] = p
                        elif fold_hold[0] is not None:
                            qd = ep.tile([128, 2, 512], BF16, tag="qf",
                                         name="qf", bufs=2)
                            nc.vector.tensor_add(
                                qd[:].rearrange("p a b -> p (a b)"),
                                fold_hold[0][:].rearrange("p a b -> p (a b)"),
                                p[:].rearrange("p a b -> p (a b)"))
                            fold_hold[0] = None
                            ones_mm(qd, last=False)
                        else:
                            ones_mm(p, last=False)
                else:
                    # diagonal chunk: causal mask, fold into the j=0 tile,
                    # single ones-MM pair once all four are in
                    w0 = 128 * j
                    mskb = msk_t[:, :, :].broadcast_to((128, 2, 128))
                    nc.vector.tensor_mul(
                        e[:, :, w0:w0 + 128], e[:, :, w0:w0 + 128], mskb)
                    if diag_acc[0] is None:
                        diag_acc[0] = e
                    else:
                        a = diag_acc[0]
                        nc.vector.tensor_add(
                            a[:, :, w0:512], a[:, :, w0:512], e[:, :, w0:512])
                    if sp:
                        ones_mm(diag_acc[0], last=True)
                w0 = max(0, 128 * j)
                nc.tensor.matmul(
                    o12[:, 0, w0:512], vb[:, kc, h * HD:(h + 1) * HD],
                    e[:, 0, w0:512], start=st, stop=sp)
                nc.tensor.matmul(
                    o12[:, 1, w0:512], vb[:, kc, h * HD:(h + 1) * HD],
                    e[:, 1, w0:512], start=st, stop=sp)

            finale_pos = min(4, nkc - 1)
            fill_pos = [max(1, nkc // 3), max(2, (2 * nkc) // 3), nkc - 1]
            fill_at = {}
            for i, f in enumerate(fillers):
                fill_at.setdefault(fill_pos[min(i, 2)], []).append(f)

            for kc in range(nkc):
                j = kc - KCQ * qc
                w0 = max(0, 128 * j)
                sch = psum.tile([128, 2, 512], F32, tag="sg", name="sch",
                                bufs=2)
                nc.tensor.matmul(
                    sch[:, 0, w0:512], kt[h][0:64, kc * 128:(kc + 1) * 128],
                    qt[h][0:64, q0 + w0:q0 + 512], start=True, stop=True)
                nc.tensor.matmul(
                    sch[:, 1, w0:512], kt[h][64:128, kc * 128:(kc + 1) * 128],
                    qt[h][64:128, q0 + w0:q0 + 512], start=True, stop=True)
                e = ep.tile([128, 2, 512], BF16, tag="e", name="e", bufs=7)
                nc.scalar.activation(
                    e[:, :, w0:512], sch[:, :, w0:512],
                    AF.Exp, scale=float(SCALING))
                pending.append((e, kc))
                if finale_prev is not None and kc == finale_pos:
                    finale_prev()
                    finale_prev = None
                for f in fill_at.get(kc, ()):
                    f()
                if len(pending) > 4:
                    emit_chunk(pending.pop(0))
            if finale_prev is not None:
                finale_prev()
            while pending:
                emit_chunk(pending.pop(0))

            # ---- epilogue (DVE part): d = (o1*s2 - lam*o2*s1)/max(|lam|,1)
            # (up to a per-column scale that the channel RMS norm washes out;
            # output sign is folded into Wo on the host).  The 1/g factor
            # keeps ssq inside ScalarE's Ln domain when lam is huge.
            # tensor_tensor can read only one PSUM operand, so s12 is staged
            # through SBUF with the 1/g scale folded in.
            s12, o12 = get_s12(), get_o12()
            s12s = at.tile([128, 2, 512], F32, tag="s12s")
            nc.vector.tensor_scalar(
                s12s[:].rearrange("p a b -> p (a b)"),
                s12[:].rearrange("p a b -> p (a b)"),
                lam_t[:, HPC + h:HPC + h + 1], None, ALU.mult)
            m = at.tile([128, 2, 512], F32, tag="m")
            nc.vector.tensor_mul(m[:], o12[:], s12s[:])
            dneg = at.tile([128, 512], BF16, tag="dneg")
            nc.vector.scalar_tensor_tensor(
                dneg[:], m[:, 1, :], lam_t[:, h:h + 1], m[:, 0, :],
                ALU.mult, ALU.subtract)
            osq = at.tile([128, 512], BF16, tag="osq")
            nc.vector.tensor_mul(osq[:], dneg[:], dneg[:])

            def finale():
                ssq = psum.tile([128, 512], F32, tag="s12", name="ssq")
                nc.tensor.matmul(ssq[:], ones_t[:], osq[:],
                                 start=True, stop=True)
                lnv = at.tile([128, 512], F32, tag="lnv")
                nc.scalar.activation(lnv[:], ssq[:], AF.Ln,
                                     scale=float(1.0 / HD), bias=eps_t[:])
                rr = at.tile([128, 512], BF16, tag="rr")
                nc.scalar.activation(rr[:], lnv[:], AF.Exp, scale=-0.5)
                nc.vector.tensor_mul(otf[h][:, q0:q0 + 512], dneg[:], rr[:])

            return finale

        # ================= pipelined schedule =================
        vp0 = psum.tile([128, 2, 512], F32, tag="s12", name="vp0")
        vp1 = psum.tile([128, 2, 512], F32, tag="o12", name="vp1")
        for mc in range(MC):
            for sc in range(4):
                t_ = vp0 if sc < 2 else vp1
                nc.tensor.matmul(
                    t_[:, sc % 2, :],
                    xtb[:, mc, sc * 128:(sc + 1) * 128],
                    wvb[:, mc, :],
                    start=(mc == 0), stop=(mc == MC - 1))
        for sc in range(4):
            t_ = vp0 if sc < 2 else vp1
            nc.vector.tensor_copy(vb[:, sc, :], t_[:, sc % 2, :])
        psA = psum.tile([128, 2, 512], F32, tag="sg", name="qkA", bufs=2)
        psB = psum.tile([128, 2, 512], F32, tag="sg", name="qkB", bufs=2)
        psC = psum.tile([128, 2, 512], F32, tag="s12", name="qkC")
        psD = psum.tile([128, 2, 512], F32, tag="o12", name="qkD")
        qk_ps = (psA, psB, psC, psD)
        for mc in range(MC):
            for hh in range(HPC):
                ps_ = qk_ps[hh]
                nc.tensor.matmul(
                    ps_[:, 0, :],
                    wqb[:, mc, hh * HD:(hh + 1) * HD],
                    xtb[:, mc, 0:512],
                    start=(mc == 0), stop=(mc == MC - 1))
                nc.tensor.matmul(
                    ps_[:, 1, :],
                    wkb[:, mc, hh * HD:(hh + 1) * HD],
                    xtb[:, mc, 0:512],
                    start=(mc == 0), stop=(mc == MC - 1))
        for hh in range(HPC):
            ps_ = qk_ps[hh]
            nc.vector.tensor_copy(qt[hh][:, 0:512], ps_[:, 0, :])
            nc.vector.tensor_copy(kt[hh][:, 0:512], ps_[:, 1, :])

        finale_prev = None
        for qc in range(QC):
            for h in range(HPC):
                fillers = []
                if qc + 1 < QC:
                    fillers.append(lambda qc=qc, h=h: proj_v(4 * (qc + 1) + h))
                    fillers.append(lambda qc=qc, h=h: proj_qk(qc + 1, h))
                if qc >= 1:
                    fillers.append(
                        lambda qc=qc, h=h: outproj2(qc - 1, 2 * h,
                                                    copy_eng=h % 2))
                finale_prev = attn_head(qc, h, finale_prev, fillers)
        finale_prev()
        for oc in range(0, 8, 2):
            outproj2(QC - 1, oc, copy_eng=(oc // 2) % 2)

    nc.compile()
    return nc


def _prep_inputs(X, Wq, Wk, Wv, Wo, lambda_q1, lambda_k1, lambda_q2,
                 lambda_k2, rms_scale):
    f32 = np.float32
    bf16 = ml_dtypes.bfloat16
    X = np.asarray(X, f32)
    Wq = np.asarray(Wq, f32)
    Wk = np.asarray(Wk, f32)
    Wv = np.asarray(Wv, f32)
    Wo = np.asarray(Wo, f32)
    lam = (np.exp(np.sum(np.asarray(lambda_q1, f32) * np.asarray(lambda_k1, f32), -1))
           - np.exp(np.sum(np.asarray(lambda_q2, f32) * np.asarray(lambda_k2, f32), -1))
           + f32(LAMBDA_INIT)).astype(f32)  # [H]
    # fold rms_scale, (1-lambda_init), and the epilogue sign flip into Wo
    wo_f = (-(Wo.reshape(H, HD, D_MODEL)
              * np.asarray(rms_scale, f32)[None, :, None]
              * f32(1.0 - LAMBDA_INIT))).astype(f32)

    # upper-triangle causal mask for diagonal 128-blocks
    kk = np.arange(128)[:, None]
    cc = np.arange(128)[None, :]
    msk = (cc >= kk).astype(f32).reshape(128, 1, 128)

    in_maps = []
    for c in range(8):
        b, hg = divmod(c, 4)
        xt = X[b].T.reshape(MC, 128, N).transpose(1, 0, 2)  # [128, MC, N]
        sl = slice(hg * HPC * HD, (hg + 1) * HPC * HD)
        wq = Wq[:, sl].reshape(MC, 128, HPC * HD).transpose(1, 0, 2)
        wk = Wk[:, sl].reshape(MC, 128, HPC * HD).transpose(1, 0, 2)
        wv = Wv[:, sl].reshape(MC, 128, HPC * HD).transpose(1, 0, 2)
        wo = wo_f[hg * HPC:(hg + 1) * HPC].reshape(HPC, HD, 8, 128).transpose(1, 0, 2, 3)
        lv = lam[hg * HPC:(hg + 1) * HPC]
        ginv = (f32(1.0) / np.maximum(np.abs(lv), f32(1.0))).astype(f32)
        lam_row = np.concatenate([lv, ginv]).astype(f32)
        lam_bc = np.broadcast_to(lam_row[None, :], (128, 2 * HPC))
        in_maps.append({
            "xt": np.ascontiguousarray(xt).astype(bf16),
            "wq": np.ascontiguousarray(wq).astype(bf16),
            "wk": np.ascontiguousarray(wk).astype(bf16),
            "wv": np.ascontiguousarray(wv).astype(bf16),
            "wo": np.ascontiguousarray(wo).astype(bf16),
            "lam": np.ascontiguousarray(lam_bc).astype(f32),
            "msk": msk.astype(bf16),
        })
    return in_maps


def kernel(X, Wq, Wk, Wv, Wo, lambda_q1, lambda_k1, lambda_q2, lambda_k2,
           rms_scale, _trace=False):
    if "nc" not in _cache:
        _cache["nc"] = _build()
    nc = _cache["nc"]
    in_maps = _prep_inputs(X, Wq, Wk, Wv, Wo, lambda_q1, lambda_k1,
                           lambda_q2, lambda_k2, rms_scale)
    res = run_bass_kernel_spmd(nc, in_maps, list(range(8)), trace=_trace)
    out = np.zeros((B, N, D_MODEL), np.float32)
    for c in range(8):
        b = c // 4
        out[b] += res.results[c]["outT"].astype(np.float32).T
    _cache["last_exec_ns"] = res.exec_time_ns
    _cache["last_res"] = res
    return out
